# revision 1
# baseline (speedup 1.0000x reference)
"""BiLSTM Trainium2 kernel.

Strategy: the LSTM recurrence is strictly sequential over T=2048 steps and is
bound by streaming W_hh (4H x H) through the PE every step.  Forward and
backward directions are independent, so core 0 runs the forward LSTM and
core 2 the backward one (different HBM stacks); the remaining cores run the
same program on zero inputs.  Each active core computes its own x-projection,
the recurrence, and its half of the fc layer; the host sums the two partial
fc outputs and adds fc_b.

Per-core device program (uniform SPMD, data differs per core):
  phase 1: xp = x @ W_ih.T + bias           -> DRAM   [T, 4H]
  phase 2: LSTM scan (For_i hardware loop)  -> DRAM hs [T, H]
  phase 3: out_partial = hs @ fcWT          -> output [T, C]

Matmul mapping for the recurrence (weights are the *moving* operand so they
stream at 1 col/cycle; fp32r = full rate):
  stationary lhsT = hT[k] (h chunk broadcast along free dim, [128,128])
  moving    rhs  = W_hhT[k-chunk, n-chunk]  ([128, 512])
  psum out [128, 512] = gates chunk replicated across partitions.
The gate chunk order i,f,g,o per h-half lets the cell update for half 0 run
on ACT/DVE while the PE computes half 1's gates.  h is returned to
partition-major layout with PE transposes for the next step's stationaries.
"""

import numpy as np

T, I, H, C = 2048, 1024, 1024, 1000
FH = 4 * H  # gate dimension
NB = FH // 512  # 8 psum-width gate chunks
KB = H // 128  # 8 contraction chunks
U = 8  # recurrence steps per For_i iteration
MM_DT = "bf16"  # "f32r" or "bf16": matmul operand dtype
ABLATE = 0  # 1: matmuls only (timing experiment; wrong numerics)

_CACHE = {}


def _split_waits(nc):
    """walrus in this toolchain rejects instructions carrying more sem waits
    than their ISA encoding has slots for ("Too many sync wait commands").
    Hoist excess waits onto injected same-engine NOPs placed just before the
    instruction (waits still all complete before it executes)."""
    import concourse.mybir as mybir

    ctr = 0
    for fn in nc.m.functions:
        for bb in fn.blocks:
            insts = bb.instructions
            if not any(
                inst.sync_info is not None
                and inst.sync_info.on_wait
                and len(inst.sync_info.on_wait) > 1
                for inst in insts
            ):
                continue
            out = []
            for inst in insts:
                si = inst.sync_info
                limit = 1
                if si is not None and si.on_wait and len(si.on_wait) > limit:
                    waits = list(si.on_wait)
                    si.on_wait = waits[len(waits) - limit:]
                    for w in waits[: len(waits) - limit]:
                        nop = mybir.InstNoOp(
                            name=f"bass-waitsplit-{ctr}",
                            engine=inst.engine,
                            ins=[],
                            outs=[],
                            sync_info=mybir.SyncInfo(on_wait=[w], on_update=[]),
                        )
                        ctr += 1
                        out.append(nop)
                out.append(inst)
            insts[:] = out


def _build(t_len):
    import concourse.bass as bass
    import concourse.mybir as mybir
    import concourse.tile as tile
    from concourse.bass import ds
    from concourse.masks import make_identity

    f32 = mybir.dt.float32
    f32r = (mybir.dt.float32r if MM_DT == "f32r" else mybir.dt.bfloat16)
    AF = mybir.ActivationFunctionType

    nc = bass.Bass()
    xT_d = nc.dram_tensor("xT", [I, t_len], f32r, kind="ExternalInput")
    wihT_d = nc.dram_tensor("wihT", [I, FH], f32r, kind="ExternalInput")
    bias_d = nc.dram_tensor("bias", [1, FH], f32r, kind="ExternalInput")
    whhT_d = nc.dram_tensor("whhT", [H, FH], f32r, kind="ExternalInput")
    fcWT_d = nc.dram_tensor("fcWT", [H, C], f32r, kind="ExternalInput")
    ones_d = nc.dram_tensor("ones1", [1, 128], f32r, kind="ExternalInput")
    zeros_d = nc.dram_tensor("zeros128", [128, 128], f32r, kind="ExternalInput")
    out_d = nc.dram_tensor("out", [t_len, C], f32, kind="ExternalOutput")

    TM = t_len // 128  # number of 128-row time tiles

    with tile.TileContext(nc) as tc:
        import contextlib

        ctx = contextlib.ExitStack()
        with ctx:
            xp_d = nc.dram_tensor("xp_scratch", [t_len, FH], f32r, kind="Internal")
            hs_d = nc.dram_tensor("hs_scratch", [t_len, H], f32, kind="Internal")

            const = ctx.enter_context(tc.tile_pool(name="const", bufs=1))
            ident = const.tile([128, 128], f32, tag="ident")
            make_identity(nc, ident[:])
            ones1 = const.tile([1, 128], f32r, tag="ones1")
            nc.sync.dma_start(ones1[:], ones_d[:, :])

            # ---------------- phase 1: xp = x @ W_ih.T + bias ----------------
            with tc.tile_pool(name="p1w", bufs=1) as p1w, \
                 tc.tile_pool(name="p1", bufs=3) as p1, \
                 tc.tile_pool(name="p1ps", bufs=4, space="PSUM") as p1ps:
                wih = []
                for k in range(KB):
                    w = p1w.tile([128, FH], f32r, tag=f"wih{k}")
                    nc.sync.dma_start(w[:], wihT_d[k * 128:(k + 1) * 128, :])
                    wih.append(w)
                bias_sb = p1w.tile([1, FH], f32r, tag="bias")
                nc.sync.dma_start(bias_sb[:], bias_d[:, :])

                for m in range(TM):
                    xt = []
                    for k in range(KB):
                        xk = p1.tile([128, 128], f32r, tag=f"xt{k}")
                        nc.sync.dma_start(
                            xk[:], xT_d[k * 128:(k + 1) * 128, m * 128:(m + 1) * 128]
                        )
                        xt.append(xk)
                    for n in range(NB):
                        ns = slice(n * 512, (n + 1) * 512)
                        ps = p1ps.tile([128, 512], f32, tag="ps")
                        nc.tensor.matmul(
                            ps[:], ones1[:], bias_sb[0:1, ns],
                            start=True, stop=False,
                        )
                        for k in range(KB):
                            nc.tensor.matmul(
                                ps[:], xt[k][:], wih[k][:, ns],
                                start=False, stop=(k == KB - 1),
                            )
                        xo = p1.tile([128, 512], f32r, tag="xo")
                        nc.scalar.copy(xo[:], ps[:])
                        nc.sync.dma_start(
                            xp_d[m * 128:(m + 1) * 128, ns], xo[:]
                        )

            tc.strict_bb_all_engine_barrier()

            # ---------------- phase 2: LSTM scan ----------------
            with tc.tile_pool(name="whh", bufs=1) as whhp, \
                 tc.tile_pool(name="state", bufs=1) as state, \
                 tc.tile_pool(name="cell", bufs=2) as cell, \
                 tc.tile_pool(name="xprow", bufs=1) as xprow_pool, \
                 tc.tile_pool(name="gps", bufs=4, space="PSUM") as gps, \
                 tc.tile_pool(name="tps", bufs=2, space="PSUM") as tps:
                whh = []
                for k in range(KB):
                    w = whhp.tile([128, FH], f32r, tag=f"whh{k}")
                    nc.sync.dma_start(w[:], whhT_d[k * 128:(k + 1) * 128, :])
                    whh.append(w)

                cst = state.tile([128, H], f32, tag="c")
                nc.vector.memset(cst[:], 0.0)
                hT = [[None] * KB for _ in range(2)]
                for p in range(2):
                    for k in range(KB):
                        ht = state.tile([128, 128], f32r, tag=f"ht{p}_{k}")
                        nc.sync.dma_start(ht[:], zeros_d[:, :])
                        hT[p][k] = ht

                with tc.For_i(
                    0, t_len, U, hint_engines=(mybir.EngineType.PE,)
                ) as iv:
                  for u in range(U):
                    tt = iv + u
                    par = u % 2  # stationary read set; write into 1 - par
                    if ABLATE == 1:
                        for n in range(NB):
                            ns = slice(n * 512, (n + 1) * 512)
                            ps = gps.tile([128, 512], f32, tag="g")
                            for k in range(KB):
                                nc.tensor.matmul(
                                    ps[:], hT[par][k][:], whh[k][:, ns],
                                    start=(k == 0), stop=(k == KB - 1),
                                )
                        continue
                    xpr = xprow_pool.tile([1, FH], f32r, tag="xpr")
                    nc.sync.dma_start(xpr[:], xp_d[ds(tt, 1), :])
                    hfull = cell.tile([128, H], f32, tag="hfull")
                    for half in range(2):
                        hsl = slice(half * 512, (half + 1) * 512)
                        gtiles = []
                        for n in (half, 2 + half, 4 + half, 6 + half):
                            ns = slice(n * 512, (n + 1) * 512)
                            ps = gps.tile([128, 512], f32, tag="g")
                            nc.tensor.matmul(
                                ps[:], ones1[:], xpr[0:1, ns],
                                start=True, stop=False,
                            )
                            for k in range(KB):
                                nc.tensor.matmul(
                                    ps[:], hT[par][k][:], whh[k][:, ns],
                                    start=False, stop=(k == KB - 1),
                                )
                            gtiles.append(ps)
                        ps_i, ps_f, ps_g, ps_o = gtiles
                        it = cell.tile([128, 512], f32, tag="it")
                        nc.scalar.activation(it[:], ps_i[:], AF.Sigmoid)
                        ft = cell.tile([128, 512], f32, tag="ft")
                        nc.scalar.activation(ft[:], ps_f[:], AF.Sigmoid)
                        gt = cell.tile([128, 512], f32, tag="gt")
                        nc.scalar.activation(gt[:], ps_g[:], AF.Tanh)
                        ot = cell.tile([128, 512], f32, tag="ot")
                        nc.scalar.activation(ot[:], ps_o[:], AF.Sigmoid)
                        ig = cell.tile([128, 512], f32, tag="ig")
                        nc.vector.tensor_mul(ig[:], it[:], gt[:])
                        fc_ = cell.tile([128, 512], f32, tag="fc")
                        nc.vector.tensor_mul(fc_[:], ft[:], cst[:, hsl])
                        nc.vector.tensor_add(cst[:, hsl], ig[:], fc_[:])
                        tcl = cell.tile([128, 512], f32, tag="tc")
                        nc.scalar.activation(tcl[:], cst[:, hsl], AF.Tanh)
                        nc.vector.tensor_mul(hfull[:, hsl], ot[:], tcl[:])
                        for q in range(4):
                            k = 4 * half + q
                            tp = tps.tile([128, 128], f32, tag="tr")
                            nc.tensor.transpose(
                                tp[:],
                                hfull[:, half * 512 + q * 128:
                                      half * 512 + (q + 1) * 128],
                                ident[:],
                            )
                            nc.scalar.copy(hT[1 - par][k][:], tp[:])
                    nc.sync.dma_start(hs_d[ds(tt, 1), :], hfull[0:1, :])

            tc.strict_bb_all_engine_barrier()

            # ---------------- phase 3: out = hs @ fcWT ----------------
            with tc.tile_pool(name="p3w", bufs=1) as p3w, \
                 tc.tile_pool(name="p3", bufs=3) as p3, \
                 tc.tile_pool(name="p3ps", bufs=2, space="PSUM") as p3ps, \
                 tc.tile_pool(name="p3tp", bufs=2, space="PSUM") as p3tp:
                fcw = []
                for k in range(KB):
                    w = p3w.tile([128, C], f32r, tag=f"fcw{k}")
                    nc.sync.dma_start(w[:], fcWT_d[k * 128:(k + 1) * 128, :])
                    fcw.append(w)
                for m in range(TM):
                    hrow = p3.tile([128, H], f32, tag="hrow")
                    nc.sync.dma_start(hrow[:], hs_d[m * 128:(m + 1) * 128, :])
                    hTt = []
                    for k in range(KB):
                        tp = p3tp.tile([128, 128], f32, tag="tr")
                        nc.tensor.transpose(
                            tp[:], hrow[:, k * 128:(k + 1) * 128], ident[:]
                        )
                        hk = p3.tile([128, 128], f32r, tag=f"hT{k}")
                        nc.scalar.copy(hk[:], tp[:])
                        hTt.append(hk)
                    for n0, nsz in ((0, 512), (512, C - 512)):
                        ps = p3ps.tile([128, nsz], f32, tag="ps")
                        for k in range(KB):
                            nc.tensor.matmul(
                                ps[:], hTt[k][:], fcw[k][:, n0:n0 + nsz],
                                start=(k == 0), stop=(k == KB - 1),
                            )
                        ob = p3.tile([128, nsz], f32, tag="ob")
                        nc.scalar.copy(ob[:], ps[:])
                        nc.sync.dma_start(
                            out_d[m * 128:(m + 1) * 128, n0:n0 + nsz], ob[:]
                        )
    _split_waits(nc)
    return nc


def _get_nc(t_len):
    if t_len not in _CACHE:
        _CACHE[t_len] = _build(t_len)
    return _CACHE[t_len]


def _mm_np_dtype():
    if MM_DT == "bf16":
        import ml_dtypes

        return ml_dtypes.bfloat16
    return np.float32


def make_in_maps(x, W_ih_f, W_hh_f, bias_f, W_ih_b, W_hh_b, bias_b, fc_W,
                 t_len):
    f = _mm_np_dtype()

    aux = {
        "ones1": np.ones((1, 128), f),
        "zeros128": np.zeros((128, 128), f),
    }

    def core_inputs(xm, wih, whh, bias, fcw):
        return {
            "xT": np.ascontiguousarray(xm.T).astype(f),
            "wihT": np.ascontiguousarray(wih.T).astype(f),
            "bias": np.ascontiguousarray(bias.reshape(1, FH)).astype(f),
            "whhT": np.ascontiguousarray(whh.T).astype(f),
            "fcWT": np.ascontiguousarray(fcw.T).astype(f),
            **aux,
        }

    zero = {
        "xT": np.zeros((I, t_len), f),
        "wihT": np.zeros((I, FH), f),
        "bias": np.zeros((1, FH), f),
        "whhT": np.zeros((H, FH), f),
        "fcWT": np.zeros((H, C), f),
        **aux,
    }
    in_maps = []
    for core in range(8):
        if core == 0:
            in_maps.append(core_inputs(x, W_ih_f, W_hh_f, bias_f, fc_W[:, :H]))
        elif core == 2:
            in_maps.append(
                core_inputs(x[::-1], W_ih_b, W_hh_b, bias_b, fc_W[:, H:])
            )
        else:
            in_maps.append(dict(zero))
    return in_maps


def _run(x, W_ih_f, W_hh_f, bias_f, W_ih_b, W_hh_b, bias_b, fc_W, t_len):
    from concourse.bass_utils import run_bass_kernel_spmd

    nc = _get_nc(t_len)
    in_maps = make_in_maps(
        x, W_ih_f, W_hh_f, bias_f, W_ih_b, W_hh_b, bias_b, fc_W, t_len
    )
    res = run_bass_kernel_spmd(nc, in_maps, core_ids=list(range(8)))
    return res.results[0]["out"] + res.results[2]["out"][::-1]


def kernel(x, W_ih_f, W_hh_f, b_ih_f, b_hh_f, W_ih_b, W_hh_b, b_ih_b, b_hh_b,
           fc_W, fc_b):
    x = np.asarray(x, np.float32)
    out = _run(
        x,
        np.asarray(W_ih_f, np.float32), np.asarray(W_hh_f, np.float32),
        np.asarray(b_ih_f, np.float32) + np.asarray(b_hh_f, np.float32),
        np.asarray(W_ih_b, np.float32), np.asarray(W_hh_b, np.float32),
        np.asarray(b_ih_b, np.float32) + np.asarray(b_hh_b, np.float32),
        np.asarray(fc_W, np.float32),
        x.shape[0],
    )
    return (out + np.asarray(fc_b, np.float32)).astype(np.float32)

